# revision 24
# baseline (speedup 1.0000x reference)
"""Causal single-head attention + RoPE on 8 TRN2 NeuronCores (Bass/Tile SPMD).

Sharding: core c -> batch b=c//2, key-parity h=c%2 (interleaved 128-key-blocks),
full Q rows. Host flash-combines the two partial softmaxes per batch.
All cores run ONE program; per-core differences (batch slice, parity) are pure
data: the sequence axis is shipped "parity-rolled" (block pairs swapped for h=1)
so the stripe's key blocks always sit at even rolled positions.

v2 pipeline:
 - phase A (per 1024-col chunk): fp16 hi/lo 3-pass QK projection, single-pass V
   projection (stripe cols only), rope on PE+DVE, hi/lo split; K stripe tensors
   and Q tensors filled by SBUF-SBUF DMAs; V transposed via DMA xbar.
 - phase B (per 128-q-block): Qh.(Kh+Kl) score matmul; mask-add + row-max fused
   in one tensor_tensor_reduce per <=512-col chunk (causal boundary masks are
   per-block data in bmask).
 - phase C (per 512-q chunk): S^T = (Kh+Kl).Qh + Kh.Ql - m with the row-max m
   and the dead-block causal masks folded into the second matmul as extra
   contraction rows; exp on scalar engine; A@V accumulated on PE. Outputs are
   written transposed [65, S]; host does the final transpose + flash combine.
B(p) groups, m round-trips and C(J) chunks are emitted interleaved so PE, DVE
and ACT pipeline across phases.
"""
import numpy as np

S, E, DH, B, P = 4096, 1024, 64, 4, 128
NEGB = -30000.0   # phase-B mask offset (phantom/tri blocks)
NEGC = -60000.0   # phase-C dead/tri mask offset
_CACHE = {}


def _build_program():
    import concourse.tile as tile
    import concourse.mybir as mybir
    from concourse import bacc
    from concourse.masks import make_identity
    from contextlib import ExitStack

    dt = mybir.dt
    f32, f16 = dt.float32, dt.float16
    AF = mybir.ActivationFunctionType
    ALU = mybir.AluOpType
    AX = mybir.AxisListType

    nc = bacc.Bacc("TRN2", target_bir_lowering=False, debug=False, num_devices=8)

    xth = nc.dram_tensor("xth", [8, P, S], f16, kind="ExternalInput").ap()
    xtl = nc.dram_tensor("xtl", [8, P, S], f16, kind="ExternalInput").ap()
    wpk = nc.dram_tensor("wpk", [P, 8, 320], f16, kind="ExternalInput").ap()
    cosr = nc.dram_tensor("cosr", [P, S], f32, kind="ExternalInput").ap()
    sinr = nc.dram_tensor("sinr", [P, S], f32, kind="ExternalInput").ap()
    p128 = nc.dram_tensor("p128", [P, P], f32, kind="ExternalInput").ap()
    bmaski = nc.dram_tensor("bmaski", [P, 32, P], f32, kind="ExternalInput").ap()
    trimi = nc.dram_tensor("trimi", [P, 256], f32, kind="ExternalInput").ap()
    dtabi = nc.dram_tensor("dtabi", [2, S], f16, kind="ExternalInput").ap()
    kseli = nc.dram_tensor("kseli", [2, S // 2], f16, kind="ExternalInput").ap()
    o_out = nc.dram_tensor("o_out", [65, S], f32, kind="ExternalOutput").ap()
    m_out = nc.dram_tensor("m_out", [32, P], f16, kind="ExternalOutput").ap()

    with tile.TileContext(nc) as tc, ExitStack() as ctx:
        const = ctx.enter_context(tc.tile_pool(name="const", bufs=1))
        xpool = ctx.enter_context(tc.tile_pool(name="xpool", bufs=2))
        work = ctx.enter_context(tc.tile_pool(name="work", bufs=2))
        wb = ctx.enter_context(tc.tile_pool(name="wb", bufs=2))
        mpool = ctx.enter_context(tc.tile_pool(name="mpool", bufs=4))
        apool = ctx.enter_context(tc.tile_pool(name="apool", bufs=3))
        psA = ctx.enter_context(tc.tile_pool(name="psA", bufs=2, space="PSUM"))
        psB = ctx.enter_context(tc.tile_pool(name="psB", bufs=2, space="PSUM"))
        psO = ctx.enter_context(tc.tile_pool(name="psO", bufs=2, space="PSUM"))
        dram = ctx.enter_context(tc.tile_pool(name="dram", bufs=1, space="DRAM"))

        # ---------- constants ----------
        w_sb = const.tile([P, 8, 320], f16)
        nc.sync.dma_start(w_sb[:], wpk[:])
        p128_sb = const.tile([P, P], f32)
        nc.sync.dma_start(p128_sb[:], p128[:])
        trim_sb = const.tile([P, 256], f32)
        nc.sync.dma_start(trim_sb[:], trimi[:])
        id32 = const.tile([P, P], f32)
        make_identity(nc, id32[:])
        id16 = const.tile([P, P], f16)
        make_identity(nc, id16[:])
        zeros512 = const.tile([P, 512], f32)
        nc.vector.memset(zeros512[:], 0.0)
        # preload the Exp activation table during phase A
        dummy = const.tile([1, 2], f16)
        nc.scalar.activation(dummy[:], zeros512[0:1, 0:2], AF.Exp)
        # phase-B partial row-max accumulators (one per key chunk index)
        Mp = const.tile([P, 4, 32], f32)
        nc.vector.memset(Mp[:], -3.0e38)

        cos_sb = const.tile([P, S], f32)
        sin_sb = const.tile([P, S], f32)
        QrHH = const.tile([P, S], f16)        # [Qh;Qh]
        QrLM = const.tile([67, S], f16)       # [Ql;m;D0;D1]
        KrHL = const.tile([P, S // 2], f16)   # [Kh;Kl] stripe
        KrH1 = const.tile([67, S // 2], f16)  # [Kh;-1;sel0;sel1]
        VT = const.tile([64, S // 2], f16)
        Vaug = const.tile([P, 16, 65], f16)
        m_sb = const.tile([P, 32], f32)
        bmask_sb = const.tile([P, 32, P], f32)

        nc.gpsimd.memset(Vaug[:, :, 64:65], 1.0)
        nc.gpsimd.memset(KrH1[64:65, :], -1.0)
        nc.gpsimd.dma_start(KrH1[65:67, :], kseli[:])
        nc.gpsimd.dma_start(QrLM[65:67, :], dtabi[:])
        nc.gpsimd.dma_start(bmask_sb[:], bmaski[:])

        # ---------- phase A: x DMAs up front (sync queue) ----------
        xhs, xls = [], []
        for u in range(4):
            cols = slice(u * 1024, (u + 1) * 1024)
            xh = xpool.tile([P, 8, 1024], f16, tag="xh")
            xl = xpool.tile([P, 8, 1024], f16, tag="xl")
            if u == 0:
                # hi parts first: the first (pass-major) projection group
                # only needs xh, so the PE starts before any xl arrives
                for ec in range(8):
                    nc.sync.dma_start(xh[:, ec, :], xth[ec, :, cols])
                for ec in range(8):
                    nc.sync.dma_start(xl[:, ec, :], xtl[ec, :, cols])
            else:
                for ec in range(8):
                    nc.sync.dma_start(xh[:, ec, :], xth[ec, :, cols])
                    nc.sync.dma_start(xl[:, ec, :], xtl[ec, :, cols])
            nc.sync.dma_start(cos_sb[:, cols], cosr[:, cols])
            nc.sync.dma_start(sin_sb[:, cols], sinr[:, cols])
            xhs.append(xh)
            xls.append(xl)

        # ---------- phase A compute (sub-range of a 1024-col chunk) ----------
        def emit_A(u, lo=0, hi=1024, pass_major=False):
            w = hi - lo
            cols = slice(u * 1024 + lo, u * 1024 + hi)
            kcols = slice((u * 1024 + lo) // 2, (u * 1024 + hi) // 2)
            xh, xl = xhs[u], xls[u]

            # QK projection: (Wh+Wl).xh + Wh.xl -> [Q(64);K(64)] x w
            # (matmul outputs are capped at one PSUM bank = 512 fp32 cols).
            # Loop ordered so consecutive matmuls reuse the stationary weights.
            halves = [slice(lo + i * 512, lo + i * 512 + 512)
                      for i in range(w // 512)]
            pk = psA.tile([P, 1024], f32, tag="big")
            if pass_major:
                # head chunks: xh-only pass first so the PE is not gated on xl
                for pi, (wc0, wc1, src) in enumerate(
                        ((0, 128, xh), (0, 128, xl), (192, 320, xh))):
                    for ec in range(8):
                        for hs in halves:
                            ps_ = slice(hs.start - lo, hs.stop - lo)
                            nc.tensor.matmul(
                                pk[:, ps_], w_sb[:, ec, wc0:wc1],
                                src[:, ec, hs],
                                start=(pi == 0 and ec == 0),
                                stop=(pi == 2 and ec == 7))
            else:
                for ec in range(8):
                    wh, wl = w_sb[:, ec, 0:128], w_sb[:, ec, 192:320]
                    for hs in halves:
                        ps_ = slice(hs.start - lo, hs.stop - lo)
                        nc.tensor.matmul(pk[:, ps_], wh, xh[:, ec, hs],
                                         start=(ec == 0), stop=False)
                    for hs in halves:
                        ps_ = slice(hs.start - lo, hs.stop - lo)
                        nc.tensor.matmul(pk[:, ps_], wh, xl[:, ec, hs],
                                         start=False, stop=False)
                    for hs in halves:
                        ps_ = slice(hs.start - lo, hs.stop - lo)
                        nc.tensor.matmul(pk[:, ps_], wl, xh[:, ec, hs],
                                         start=False, stop=(ec == 7))
            traw = work.tile([P, 1024], f32, tag="traw")
            nc.scalar.copy(traw[:, 0:w], pk[:, 0:w])

            # V projection on stripe cols (rolled-even blocks) of this range
            vps = psB.tile([64, 512], f32, tag="half")
            xh_blk = xh.rearrange("p e (n two c) -> p e n two c", two=2, c=128)
            n0, n1 = lo // 256, hi // 256
            for ec in range(8):
                nc.tensor.matmul(vps[:, 0:w // 2], w_sb[:, ec, 128:192],
                                 xh_blk[:, ec, n0:n1, 0, :],
                                 start=(ec == 0), stop=(ec == 7))
            nc.scalar.copy(VT[:, kcols], vps[:, 0:w // 2])

            # rope: r = traw*cos + swap(traw)*sin
            tss = []
            for i, hs in enumerate(halves):
                ts = psB.tile([P, 512], f32, tag="half")
                nc.tensor.matmul(ts[:], p128_sb[:],
                                 traw[:, i * 512:i * 512 + 512],
                                 start=True, stop=True)
                tss.append(ts)
            t1 = work.tile([P, 1024], f32, tag="t1")
            nc.vector.tensor_tensor(t1[:, 0:w], traw[:, 0:w], cos_sb[:, cols], ALU.mult)
            r = work.tile([P, 1024], f32, tag="r")
            for i, hs in enumerate(halves):
                nc.vector.tensor_tensor(
                    r[:, i * 512:i * 512 + 512], tss[i][:],
                    sin_sb[:, u * 1024 + hs.start:u * 1024 + hs.stop], ALU.mult)
            nc.vector.tensor_tensor(r[:, 0:w], r[:, 0:w], t1[:, 0:w], ALU.add)
            rh = work.tile([P, 1024], f16, tag="rh")
            nc.vector.tensor_copy(rh[:, 0:w], r[:, 0:w])
            rh32 = work.tile([P, 1024], f32, tag="rh32")
            nc.scalar.copy(rh32[:, 0:w], rh[:, 0:w])
            rl = work.tile([P, 1024], f16, tag="rl")
            nc.vector.tensor_tensor(rl[:, 0:w], r[:, 0:w], rh32[:, 0:w], ALU.subtract)

            # distribute (SBUF-SBUF DMAs on gpsimd queue)
            rh_ev = rh[64:128, 0:w].rearrange("p (n two c) -> p n two c", two=2, c=128)[:, :, 0, :]
            rl_ev = rl[64:128, 0:w].rearrange("p (n two c) -> p n two c", two=2, c=128)[:, :, 0, :]
            kh_dst = KrHL[0:64, kcols].rearrange("p (n c) -> p n c", c=128)
            kl_dst = KrHL[64:128, kcols].rearrange("p (n c) -> p n c", c=128)
            k1_dst = KrH1[0:64, kcols].rearrange("p (n c) -> p n c", c=128)
            nc.gpsimd.dma_start(kh_dst, rh_ev)
            nc.gpsimd.dma_start(kl_dst, rl_ev)
            nc.gpsimd.dma_start(k1_dst, rh_ev)
            nc.gpsimd.dma_start(QrHH[0:64, cols], rh[0:64, 0:w])
            nc.gpsimd.dma_start(QrHH[64:128, cols], rh[0:64, 0:w])
            nc.gpsimd.dma_start(QrLM[0:64, cols], rl[0:64, 0:w])

            # Vaug: PE transposes of VT blocks
            for t in range(w // 256):
                blk = (u * 1024 + lo) // 256 + t
                vt_ps = psO.tile([P, 64], f16, tag="o")
                nc.tensor.transpose(vt_ps[:], VT[:, blk * P:(blk + 1) * P],
                                    id16[0:64, 0:64])
                nc.vector.tensor_copy(Vaug[:, blk, 0:64], vt_ps[:])

        # ---------- phases B and C, interleaved ----------
        def emit_B_group(g):
            for p in range(4 * g, 4 * g + 4):
                cu = p // 2 + 1
                nch = (cu + 3) // 4
                lhs = QrHH[:, p * P:(p + 1) * P]
                for ch in range(nch):
                    w = min(4, cu - 4 * ch)
                    wid = w * P
                    spb = psB.tile([P, 512], f32, tag="half")
                    nc.tensor.matmul(spb[:, :wid], lhs,
                                     KrHL[:, ch * 512:ch * 512 + wid],
                                     start=True, stop=True)
                    if ch == nch - 1:  # causal boundary mask on the last block
                        nc.vector.tensor_tensor(
                            spb[:, wid - P:wid], spb[:, wid - P:wid],
                            bmask_sb[:, p, :], ALU.add)
                    nc.vector.reduce_max(Mp[:, ch, p:p + 1], spb[:, :wid],
                                         axis=AX.X)
            c0, c1 = slice(4 * g, 4 * g + 4), slice(4 * g, 4 * g + 4)
            ma = mpool.tile([P, 4], f32, tag="mt")
            nc.vector.tensor_tensor(ma[:], Mp[:, 0, c0], Mp[:, 1, c0], ALU.max)
            nc.vector.tensor_tensor(ma[:], ma[:], Mp[:, 2, c0], ALU.max)
            nc.vector.tensor_tensor(m_sb[:, c0], ma[:], Mp[:, 3, c0], ALU.max)

        m_dr = dram.tile([8, 4, P], f16)

        def emit_m(J):
            mtp = psO.tile([65, 512], f32, tag="o")
            nc.tensor.transpose(mtp[0:4, 0:128], m_sb[:, 4 * J:4 * J + 4], id32[:])
            mrow = wb.tile([4, P], f16, tag="mrow")
            nc.vector.tensor_copy(mrow[:], mtp[0:4, 0:128])
            nc.sync.dma_start(m_dr[J], mrow[:])
            nc.sync.dma_start(m_out[4 * J:4 * J + 4, :], mrow[:])
            nc.sync.dma_start(QrLM[64:65, 512 * J:512 * (J + 1)],
                              m_dr[J].rearrange("a b -> (a b)")[None, :])

        def emit_C(J):
            qc = slice(512 * J, 512 * (J + 1))
            ops = psO.tile([65, 512], f32, tag="o")
            a_prev = None

            def emit_av(jj, a):
                nc.tensor.matmul(ops[:], Vaug[:, 2 * jj, 0:65], a[:, 0:512],
                                 start=(jj == 0), stop=False)
                nc.tensor.matmul(ops[:], Vaug[:, 2 * jj + 1, 0:65], a[:, 512:1024],
                                 start=False, stop=(jj == J))

            for jj in range(J + 1):
                boundary = (jj == J)
                sp = psA.tile([P, 1024], f32, tag="big")
                for t in range(2):
                    j = 2 * jj + t
                    kb = slice(j * P, (j + 1) * P)
                    half = slice(512 * t, 512 * t + 512)
                    nc.tensor.matmul(sp[:, half], KrHL[:, kb], QrHH[:, qc],
                                     start=True, stop=False)
                    hi = 67 if boundary else 65
                    nc.tensor.matmul(sp[:, half], KrH1[0:hi, kb], QrLM[0:hi, qc],
                                     start=False, stop=True)
                if a_prev is not None:
                    emit_av(jj - 1, a_prev)
                if boundary:
                    nc.vector.tensor_tensor(sp[:, 0:256], sp[:, 0:256],
                                            trim_sb[:], ALU.add)
                    nc.vector.tensor_tensor(sp[:, 768:1024], sp[:, 768:1024],
                                            trim_sb[:], ALU.add)
                a = apool.tile([P, 1024], f16, tag="a")
                nc.scalar.activation(a[:], sp[:], AF.Exp)
                a_prev = a
            emit_av(J, a_prev)
            osb = wb.tile([65, 512], f32, tag="osb")
            nc.vector.tensor_copy(osb[:], ops[:])
            nc.sync.dma_start(o_out[:, qc], osb[:])

        # interleaved schedule: B groups ride phase A's vector-idle window;
        # each C(J) is emitted well after its m(J) round-trip was issued.
        emit_A(0, 0, 512, pass_major=True)
        emit_A(0, 512, 1024, pass_major=True)
        emit_A(1)
        emit_B_group(0)
        emit_B_group(1)
        emit_m(0)
        emit_A(2)
        emit_B_group(2)
        emit_B_group(3)
        emit_m(1)
        emit_C(0)
        emit_A(3)
        emit_B_group(4)
        emit_B_group(5)
        emit_m(2)
        emit_C(1)
        emit_B_group(6)
        emit_m(3)
        emit_C(2)
        emit_B_group(7)
        emit_m(4)
        emit_C(3)
        emit_m(5)
        emit_C(4)
        emit_m(6)
        emit_C(5)
        emit_m(7)
        emit_C(6)
        emit_C(7)

    nc.compile()
    return nc


def _roll_order(h):
    o = np.arange(32).reshape(16, 2)
    return (o if h == 0 else o[:, ::-1]).reshape(-1)


def _host_prep(x, W_Q, W_K, W_V):
    perm = np.empty(64, np.int64)
    perm[:32] = np.arange(32) * 2
    perm[32:] = np.arange(32) * 2 + 1
    wq = (np.asarray(W_Q, np.float64)[:, perm] / 8.0)
    wk = np.asarray(W_K, np.float64)[:, perm]

    def sp(w):
        h = w.astype(np.float16)
        return h, (w - h.astype(np.float64)).astype(np.float16)

    wqh, wql = sp(wq)
    wkh, wkl = sp(wk)
    wvh = np.asarray(W_V, np.float16)
    wpk = np.concatenate([wqh, wkh, wvh, wql, wkl], axis=1)
    wpk = np.ascontiguousarray(wpk.reshape(8, 128, 320).transpose(1, 0, 2))

    pos = np.arange(S, dtype=np.float64)
    inv = 1.0 / (10000.0 ** (2.0 * np.arange(32) / 64.0))
    th = pos[None, :] * inv[:, None]
    c64 = np.concatenate([np.cos(th), np.cos(th)], 0)
    s64 = np.concatenate([-np.sin(th), np.sin(th)], 0)
    cos2 = np.concatenate([c64, c64], 0).astype(np.float32)
    sin2 = np.concatenate([s64, s64], 0).astype(np.float32)

    p128 = np.zeros((P, P), np.float32)
    for d in range(32):
        for base in (0, 64):
            p128[base + d + 32, base + d] = 1.0
            p128[base + d, base + d + 32] = 1.0

    qi = np.arange(P)[:, None]   # q row within block
    ki = np.arange(P)[None, :]   # k col within block
    tri_b = np.where(ki > qi, NEGB, 0.0).astype(np.float32)   # [q, k]
    negb = np.full((P, P), NEGB, np.float32)
    zb = np.zeros((P, P), np.float32)

    kr = np.arange(P)[:, None]   # k row
    qr = np.arange(P)[None, :]   # q col
    tri_c = np.where(kr > qr, NEGC, 0.0).astype(np.float32)   # [k, q]
    negc = np.full((P, P), NEGC, np.float32)

    cos_h, sin_h, bmask_h, trim_h, dtab_h = [], [], [], [], []
    for h in (0, 1):
        order = _roll_order(h)
        colidx = (order[:, None] * P + np.arange(P)[None, :]).reshape(-1)
        cos_h.append(np.ascontiguousarray(cos2[:, colidx]))
        sin_h.append(np.ascontiguousarray(sin2[:, colidx]))

        bm = np.empty((32, P, P), np.float32)
        for p in range(32):
            if p % 2 == 0:
                bm[p] = tri_b
            else:
                bm[p] = zb if h == 0 else negb
        bmask_h.append(np.ascontiguousarray(bm.transpose(1, 0, 2)))

        tr = np.concatenate([tri_c, zb if h == 0 else negc], axis=1)
        trim_h.append(np.ascontiguousarray(tr.astype(np.float32)))

        qb4 = (np.arange(S) // P) % 4
        d0 = ((qb4 == 1) if h == 1 else np.zeros(S, bool))
        d1 = (qb4 <= 1)
        dtab_h.append(np.ascontiguousarray(
            np.stack([d0, d1]).astype(np.float16)))

    r2 = (np.arange(S // 2) // P) % 2
    ksel = np.stack([np.where(r2 == 0, NEGC, 0.0),
                     np.where(r2 == 1, NEGC, 0.0)]).astype(np.float16)

    xts = []
    x64 = np.asarray(x, np.float64)
    for h in (0, 1):
        order = _roll_order(h)
        xr = x64.transpose(0, 2, 1).reshape(B, 8, 128, 32, 128)[:, :, :, order, :]
        xr = xr.reshape(B, 8, 128, S)
        xh = xr.astype(np.float16)
        xl = (xr - xh.astype(np.float64)).astype(np.float16)
        xts.append((np.ascontiguousarray(xh), np.ascontiguousarray(xl)))
    return dict(wpk=wpk, p128=p128, cos_h=cos_h, sin_h=sin_h, bmask_h=bmask_h,
                trim_h=trim_h, dtab_h=dtab_h, ksel=ksel, xts=xts)


def _in_maps(prep):
    ms = []
    for c in range(8):
        b, h = c // 2, c % 2
        xh, xl = prep["xts"][h]
        ms.append({
            "xth": xh[b], "xtl": xl[b], "wpk": prep["wpk"],
            "cosr": prep["cos_h"][h], "sinr": prep["sin_h"][h],
            "p128": prep["p128"], "bmaski": prep["bmask_h"][h],
            "trimi": prep["trim_h"][h], "dtabi": prep["dtab_h"][h],
            "kseli": prep["ksel"],
        })
    return ms


def kernel(x, mask, W_Q, W_K, W_V):
    x = np.asarray(x, np.float32)
    mask = np.asarray(mask)
    if not np.array_equal(mask, np.tril(np.ones((S, S), mask.dtype))):
        Q = x @ W_Q
        K = x @ W_K
        V = x @ W_V
        pos = np.arange(S)
        inv = 1.0 / (10000.0 ** (2.0 * np.arange(32) / 64))
        th = pos[:, None] * inv[None, :]
        sn, cs = np.sin(th), np.cos(th)

        def rp(q):
            x1, x2 = q[..., 0::2], q[..., 1::2]
            o = np.empty_like(q)
            o[..., 0::2] = x1 * cs - x2 * sn
            o[..., 1::2] = x1 * sn + x2 * cs
            return o
        Q, K = rp(Q), rp(K)
        s = np.einsum('bqd,bkd->bqk', Q, K) / 8.0
        s = np.where(mask == 0, -np.inf, s)
        e = np.exp(s - s.max(-1, keepdims=True))
        return (np.einsum('bqk,bkd->bqd', e / e.sum(-1, keepdims=True), V)
                ).astype(np.float32)

    from concourse.bass_utils import run_bass_kernel_spmd
    if "nc" not in _CACHE:
        _CACHE["nc"] = _build_program()
    nc = _CACHE["nc"]

    prep = _host_prep(x, W_Q, W_K, W_V)
    res = run_bass_kernel_spmd(nc, _in_maps(prep), core_ids=list(range(8)))

    out = np.empty((B, S, DH), np.float32)
    for b in range(B):
        oa, mm = [], []
        for h in (0, 1):
            rr = res.results[2 * b + h]
            order = _roll_order(h)
            unroll = np.empty(32, np.int64)
            unroll[order] = np.arange(32)   # global block -> rolled position
            o_r = rr["o_out"].astype(np.float64).reshape(65, 32, 128)[:, unroll]
            m_r = rr["m_out"].astype(np.float64)[unroll]
            oa.append(o_r.reshape(65, S))
            mm.append(m_r.reshape(S))
        M = np.maximum(mm[0], mm[1])
        w = [np.exp(np.minimum(mm[h] - M, 0.0)) for h in (0, 1)]
        num = oa[0][:64] * w[0][None, :] + oa[1][:64] * w[1][None, :]
        den = oa[0][64] * w[0] + oa[1][64] * w[1]
        out[b] = (num / den[None, :]).T.astype(np.float32)
    return out


# revision 25
# speedup vs baseline: 1.0475x; 1.0475x over previous
"""Causal single-head attention + RoPE on 8 TRN2 NeuronCores (Bass/Tile SPMD).

Sharding: core c -> batch b=c//2, key-parity h=c%2 (interleaved 128-key-blocks),
full Q rows. Host flash-combines the two partial softmaxes per batch.
All cores run ONE program; per-core differences (batch slice, parity) are pure
data: the sequence axis is shipped "parity-rolled" (block pairs swapped for h=1)
so the stripe's key blocks always sit at even rolled positions.

v2 pipeline:
 - phase A (per 1024-col chunk): fp16 hi/lo 3-pass QK projection, single-pass V
   projection (stripe cols only), rope on PE+DVE, hi/lo split; K stripe tensors
   and Q tensors filled by SBUF-SBUF DMAs; V transposed via DMA xbar.
 - phase B (per 128-q-block): Qh.(Kh+Kl) score matmul; mask-add + row-max fused
   in one tensor_tensor_reduce per <=512-col chunk (causal boundary masks are
   per-block data in bmask).
 - phase C (per 512-q chunk): S^T = (Kh+Kl).Qh + Kh.Ql - m with the row-max m
   and the dead-block causal masks folded into the second matmul as extra
   contraction rows; exp on scalar engine; A@V accumulated on PE. Outputs are
   written transposed [65, S]; host does the final transpose + flash combine.
B(p) groups, m round-trips and C(J) chunks are emitted interleaved so PE, DVE
and ACT pipeline across phases.
"""
import numpy as np

S, E, DH, B, P = 4096, 1024, 64, 4, 128
NEGB = -30000.0   # phase-B mask offset (phantom/tri blocks)
NEGC = -60000.0   # phase-C dead/tri mask offset
_CACHE = {}


def _build_program():
    import concourse.tile as tile
    import concourse.mybir as mybir
    from concourse import bacc
    from concourse.masks import make_identity
    from contextlib import ExitStack

    dt = mybir.dt
    f32, f16 = dt.float32, dt.float16
    AF = mybir.ActivationFunctionType
    ALU = mybir.AluOpType
    AX = mybir.AxisListType

    nc = bacc.Bacc("TRN2", target_bir_lowering=False, debug=False, num_devices=8)

    xth = nc.dram_tensor("xth", [8, P, S], f16, kind="ExternalInput").ap()
    xtl = nc.dram_tensor("xtl", [8, P, S], f16, kind="ExternalInput").ap()
    wpk = nc.dram_tensor("wpk", [P, 8, 320], f16, kind="ExternalInput").ap()
    cosr = nc.dram_tensor("cosr", [P, S], f32, kind="ExternalInput").ap()
    sinr = nc.dram_tensor("sinr", [P, S], f32, kind="ExternalInput").ap()
    p128 = nc.dram_tensor("p128", [P, P], f32, kind="ExternalInput").ap()
    bmaski = nc.dram_tensor("bmaski", [P, 32, P], f32, kind="ExternalInput").ap()
    trimi = nc.dram_tensor("trimi", [P, 256], f32, kind="ExternalInput").ap()
    dtabi = nc.dram_tensor("dtabi", [2, S], f16, kind="ExternalInput").ap()
    kseli = nc.dram_tensor("kseli", [2, S // 2], f16, kind="ExternalInput").ap()
    o_out = nc.dram_tensor("o_out", [65, S], f32, kind="ExternalOutput").ap()
    m_out = nc.dram_tensor("m_out", [32, P], f16, kind="ExternalOutput").ap()

    with tile.TileContext(nc) as tc, ExitStack() as ctx:
        const = ctx.enter_context(tc.tile_pool(name="const", bufs=1))
        xpool = ctx.enter_context(tc.tile_pool(name="xpool", bufs=2))
        work = ctx.enter_context(tc.tile_pool(name="work", bufs=2))
        wb = ctx.enter_context(tc.tile_pool(name="wb", bufs=2))
        mpool = ctx.enter_context(tc.tile_pool(name="mpool", bufs=4))
        apool = ctx.enter_context(tc.tile_pool(name="apool", bufs=3))
        psA = ctx.enter_context(tc.tile_pool(name="psA", bufs=2, space="PSUM"))
        psB = ctx.enter_context(tc.tile_pool(name="psB", bufs=2, space="PSUM"))
        psO = ctx.enter_context(tc.tile_pool(name="psO", bufs=2, space="PSUM"))
        dram = ctx.enter_context(tc.tile_pool(name="dram", bufs=1, space="DRAM"))

        # ---------- constants ----------
        w_sb = const.tile([P, 8, 320], f16)
        nc.sync.dma_start(w_sb[:], wpk[:])
        p128_sb = const.tile([P, P], f32)
        nc.sync.dma_start(p128_sb[:], p128[:])
        trim_sb = const.tile([P, 256], f32)
        nc.sync.dma_start(trim_sb[:], trimi[:])
        id32 = const.tile([P, P], f32)
        make_identity(nc, id32[:])
        id16 = const.tile([P, P], f16)
        make_identity(nc, id16[:])
        zeros512 = const.tile([P, 512], f32)
        nc.vector.memset(zeros512[:], 0.0)
        # preload the Exp activation table during phase A
        dummy = const.tile([1, 2], f16)
        nc.scalar.activation(dummy[:], zeros512[0:1, 0:2], AF.Exp)
        # phase-B partial row-max accumulators (one per key chunk index)
        Mp = const.tile([P, 4, 32], f32)
        nc.vector.memset(Mp[:], -3.0e38)

        cos_sb = const.tile([P, S], f32)
        sin_sb = const.tile([P, S], f32)
        QrHH = const.tile([P, S], f16)        # [Qh;Qh]
        QrLM = const.tile([67, S], f16)       # [Ql;m;D0;D1]
        KrHL = const.tile([P, S // 2], f16)   # [Kh;Kl] stripe
        KrH1 = const.tile([67, S // 2], f16)  # [Kh;-1;sel0;sel1]
        VT = const.tile([64, S // 2], f16)
        Vaug = const.tile([P, 16, 65], f16)
        m_sb = const.tile([P, 32], f32)
        bmask_sb = const.tile([P, 32, P], f32)

        nc.gpsimd.memset(Vaug[:, :, 64:65], 1.0)
        nc.gpsimd.memset(KrH1[64:65, :], -1.0)
        nc.gpsimd.dma_start(KrH1[65:67, :], kseli[:])
        nc.gpsimd.dma_start(QrLM[65:67, :], dtabi[:])
        nc.gpsimd.dma_start(bmask_sb[:], bmaski[:])

        # ---------- phase A: x DMAs up front (sync queue) ----------
        xhs, xls = [], []
        for u in range(4):
            cols = slice(u * 1024, (u + 1) * 1024)
            xh = xpool.tile([P, 8, 1024], f16, tag="xh")
            xl = xpool.tile([P, 8, 1024], f16, tag="xl")
            for ec in range(8):
                nc.sync.dma_start(xh[:, ec, :], xth[ec, :, cols])
                nc.sync.dma_start(xl[:, ec, :], xtl[ec, :, cols])
            nc.sync.dma_start(cos_sb[:, cols], cosr[:, cols])
            nc.sync.dma_start(sin_sb[:, cols], sinr[:, cols])
            xhs.append(xh)
            xls.append(xl)

        # ---------- phase A compute (sub-range of a 1024-col chunk) ----------
        def emit_A(u, lo=0, hi=1024):
            w = hi - lo
            cols = slice(u * 1024 + lo, u * 1024 + hi)
            kcols = slice((u * 1024 + lo) // 2, (u * 1024 + hi) // 2)
            xh, xl = xhs[u], xls[u]

            # QK projection: (Wh+Wl).xh + Wh.xl -> [Q(64);K(64)] x w
            # (matmul outputs are capped at one PSUM bank = 512 fp32 cols).
            # Loop ordered so consecutive matmuls reuse the stationary weights.
            halves = [slice(lo + i * 512, lo + i * 512 + 512)
                      for i in range(w // 512)]
            pk = psA.tile([P, 1024], f32, tag="big")
            for ec in range(8):
                wh, wl = w_sb[:, ec, 0:128], w_sb[:, ec, 192:320]
                for hs in halves:
                    ps_ = slice(hs.start - lo, hs.stop - lo)
                    nc.tensor.matmul(pk[:, ps_], wh, xh[:, ec, hs],
                                     start=(ec == 0), stop=False)
                for hs in halves:
                    ps_ = slice(hs.start - lo, hs.stop - lo)
                    nc.tensor.matmul(pk[:, ps_], wh, xl[:, ec, hs],
                                     start=False, stop=False)
                for hs in halves:
                    ps_ = slice(hs.start - lo, hs.stop - lo)
                    nc.tensor.matmul(pk[:, ps_], wl, xh[:, ec, hs],
                                     start=False, stop=(ec == 7))
            traw = work.tile([P, 1024], f32, tag="traw")
            nc.scalar.copy(traw[:, 0:w], pk[:, 0:w])

            # V projection on stripe cols (rolled-even blocks) of this range
            vps = psB.tile([64, 512], f32, tag="half")
            xh_blk = xh.rearrange("p e (n two c) -> p e n two c", two=2, c=128)
            n0, n1 = lo // 256, hi // 256
            for ec in range(8):
                nc.tensor.matmul(vps[:, 0:w // 2], w_sb[:, ec, 128:192],
                                 xh_blk[:, ec, n0:n1, 0, :],
                                 start=(ec == 0), stop=(ec == 7))
            nc.scalar.copy(VT[:, kcols], vps[:, 0:w // 2])

            # rope: r = traw*cos + swap(traw)*sin
            tss = []
            for i, hs in enumerate(halves):
                ts = psB.tile([P, 512], f32, tag="half")
                nc.tensor.matmul(ts[:], p128_sb[:],
                                 traw[:, i * 512:i * 512 + 512],
                                 start=True, stop=True)
                tss.append(ts)
            t1 = work.tile([P, 1024], f32, tag="t1")
            nc.vector.tensor_tensor(t1[:, 0:w], traw[:, 0:w], cos_sb[:, cols], ALU.mult)
            r = work.tile([P, 1024], f32, tag="r")
            for i, hs in enumerate(halves):
                nc.vector.tensor_tensor(
                    r[:, i * 512:i * 512 + 512], tss[i][:],
                    sin_sb[:, u * 1024 + hs.start:u * 1024 + hs.stop], ALU.mult)
            nc.vector.tensor_tensor(r[:, 0:w], r[:, 0:w], t1[:, 0:w], ALU.add)
            rh = work.tile([P, 1024], f16, tag="rh")
            nc.vector.tensor_copy(rh[:, 0:w], r[:, 0:w])
            rh32 = work.tile([P, 1024], f32, tag="rh32")
            nc.scalar.copy(rh32[:, 0:w], rh[:, 0:w])
            rl = work.tile([P, 1024], f16, tag="rl")
            nc.vector.tensor_tensor(rl[:, 0:w], r[:, 0:w], rh32[:, 0:w], ALU.subtract)

            # distribute (SBUF-SBUF DMAs on gpsimd queue)
            rh_ev = rh[64:128, 0:w].rearrange("p (n two c) -> p n two c", two=2, c=128)[:, :, 0, :]
            rl_ev = rl[64:128, 0:w].rearrange("p (n two c) -> p n two c", two=2, c=128)[:, :, 0, :]
            kh_dst = KrHL[0:64, kcols].rearrange("p (n c) -> p n c", c=128)
            kl_dst = KrHL[64:128, kcols].rearrange("p (n c) -> p n c", c=128)
            k1_dst = KrH1[0:64, kcols].rearrange("p (n c) -> p n c", c=128)
            nc.gpsimd.dma_start(kh_dst, rh_ev)
            nc.gpsimd.dma_start(kl_dst, rl_ev)
            nc.gpsimd.dma_start(k1_dst, rh_ev)
            nc.gpsimd.dma_start(QrHH[0:64, cols], rh[0:64, 0:w])
            nc.gpsimd.dma_start(QrHH[64:128, cols], rh[0:64, 0:w])
            nc.gpsimd.dma_start(QrLM[0:64, cols], rl[0:64, 0:w])

            # Vaug: PE transposes of VT blocks
            for t in range(w // 256):
                blk = (u * 1024 + lo) // 256 + t
                vt_ps = psO.tile([P, 64], f16, tag="o")
                nc.tensor.transpose(vt_ps[:], VT[:, blk * P:(blk + 1) * P],
                                    id16[0:64, 0:64])
                nc.vector.tensor_copy(Vaug[:, blk, 0:64], vt_ps[:])

        # ---------- phases B and C, interleaved ----------
        def emit_B_group(g):
            for p in range(4 * g, 4 * g + 4):
                cu = p // 2 + 1
                nch = (cu + 3) // 4
                lhs = QrHH[:, p * P:(p + 1) * P]
                for ch in range(nch):
                    w = min(4, cu - 4 * ch)
                    wid = w * P
                    spb = psB.tile([P, 512], f32, tag="half")
                    nc.tensor.matmul(spb[:, :wid], lhs,
                                     KrHL[:, ch * 512:ch * 512 + wid],
                                     start=True, stop=True)
                    if ch == nch - 1:  # causal boundary mask on the last block
                        nc.vector.tensor_tensor(
                            spb[:, wid - P:wid], spb[:, wid - P:wid],
                            bmask_sb[:, p, :], ALU.add)
                    nc.vector.reduce_max(Mp[:, ch, p:p + 1], spb[:, :wid],
                                         axis=AX.X)
            c0, c1 = slice(4 * g, 4 * g + 4), slice(4 * g, 4 * g + 4)
            ma = mpool.tile([P, 4], f32, tag="mt")
            nc.vector.tensor_tensor(ma[:], Mp[:, 0, c0], Mp[:, 1, c0], ALU.max)
            nc.vector.tensor_tensor(ma[:], ma[:], Mp[:, 2, c0], ALU.max)
            nc.vector.tensor_tensor(m_sb[:, c0], ma[:], Mp[:, 3, c0], ALU.max)

        m_dr = dram.tile([8, 4, P], f16)

        def emit_m(J):
            mtp = psO.tile([65, 512], f32, tag="o")
            nc.tensor.transpose(mtp[0:4, 0:128], m_sb[:, 4 * J:4 * J + 4], id32[:])
            mrow = wb.tile([4, P], f16, tag="mrow")
            nc.vector.tensor_copy(mrow[:], mtp[0:4, 0:128])
            nc.sync.dma_start(m_dr[J], mrow[:])
            nc.sync.dma_start(m_out[4 * J:4 * J + 4, :], mrow[:])
            nc.sync.dma_start(QrLM[64:65, 512 * J:512 * (J + 1)],
                              m_dr[J].rearrange("a b -> (a b)")[None, :])

        def emit_C(J):
            qc = slice(512 * J, 512 * (J + 1))
            ops = psO.tile([65, 512], f32, tag="o")
            a_prev = None

            def emit_av(jj, a):
                nc.tensor.matmul(ops[:], Vaug[:, 2 * jj, 0:65], a[:, 0:512],
                                 start=(jj == 0), stop=False)
                nc.tensor.matmul(ops[:], Vaug[:, 2 * jj + 1, 0:65], a[:, 512:1024],
                                 start=False, stop=(jj == J))

            for jj in range(J + 1):
                boundary = (jj == J)
                sp = psA.tile([P, 1024], f32, tag="big")
                for t in range(2):
                    j = 2 * jj + t
                    kb = slice(j * P, (j + 1) * P)
                    half = slice(512 * t, 512 * t + 512)
                    nc.tensor.matmul(sp[:, half], KrHL[:, kb], QrHH[:, qc],
                                     start=True, stop=False)
                    hi = 67 if boundary else 65
                    nc.tensor.matmul(sp[:, half], KrH1[0:hi, kb], QrLM[0:hi, qc],
                                     start=False, stop=True)
                if a_prev is not None:
                    emit_av(jj - 1, a_prev)
                if boundary:
                    nc.vector.tensor_tensor(sp[:, 0:256], sp[:, 0:256],
                                            trim_sb[:], ALU.add)
                    nc.vector.tensor_tensor(sp[:, 768:1024], sp[:, 768:1024],
                                            trim_sb[:], ALU.add)
                a = apool.tile([P, 1024], f16, tag="a")
                nc.scalar.activation(a[:], sp[:], AF.Exp)
                a_prev = a
            emit_av(J, a_prev)
            osb = wb.tile([65, 512], f32, tag="osb")
            nc.vector.tensor_copy(osb[:], ops[:])
            nc.sync.dma_start(o_out[:, qc], osb[:])

        # interleaved schedule: B groups ride phase A's vector-idle window;
        # each C(J) is emitted well after its m(J) round-trip was issued.
        emit_A(0, 0, 512)
        emit_A(0, 512, 1024)
        emit_A(1)
        emit_B_group(0)
        emit_B_group(1)
        emit_m(0)
        emit_A(2)
        emit_B_group(2)
        emit_B_group(3)
        emit_m(1)
        emit_C(0)
        emit_A(3)
        emit_B_group(4)
        emit_B_group(5)
        emit_m(2)
        emit_C(1)
        emit_B_group(6)
        emit_m(3)
        emit_C(2)
        emit_B_group(7)
        emit_m(4)
        emit_C(3)
        emit_m(5)
        emit_C(4)
        emit_m(6)
        emit_C(5)
        emit_m(7)
        emit_C(6)
        emit_C(7)

    nc.compile()
    return nc


def _roll_order(h):
    o = np.arange(32).reshape(16, 2)
    return (o if h == 0 else o[:, ::-1]).reshape(-1)


def _host_prep(x, W_Q, W_K, W_V):
    perm = np.empty(64, np.int64)
    perm[:32] = np.arange(32) * 2
    perm[32:] = np.arange(32) * 2 + 1
    wq = (np.asarray(W_Q, np.float64)[:, perm] / 8.0)
    wk = np.asarray(W_K, np.float64)[:, perm]

    def sp(w):
        h = w.astype(np.float16)
        return h, (w - h.astype(np.float64)).astype(np.float16)

    wqh, wql = sp(wq)
    wkh, wkl = sp(wk)
    wvh = np.asarray(W_V, np.float16)
    wpk = np.concatenate([wqh, wkh, wvh, wql, wkl], axis=1)
    wpk = np.ascontiguousarray(wpk.reshape(8, 128, 320).transpose(1, 0, 2))

    pos = np.arange(S, dtype=np.float64)
    inv = 1.0 / (10000.0 ** (2.0 * np.arange(32) / 64.0))
    th = pos[None, :] * inv[:, None]
    c64 = np.concatenate([np.cos(th), np.cos(th)], 0)
    s64 = np.concatenate([-np.sin(th), np.sin(th)], 0)
    cos2 = np.concatenate([c64, c64], 0).astype(np.float32)
    sin2 = np.concatenate([s64, s64], 0).astype(np.float32)

    p128 = np.zeros((P, P), np.float32)
    for d in range(32):
        for base in (0, 64):
            p128[base + d + 32, base + d] = 1.0
            p128[base + d, base + d + 32] = 1.0

    qi = np.arange(P)[:, None]   # q row within block
    ki = np.arange(P)[None, :]   # k col within block
    tri_b = np.where(ki > qi, NEGB, 0.0).astype(np.float32)   # [q, k]
    negb = np.full((P, P), NEGB, np.float32)
    zb = np.zeros((P, P), np.float32)

    kr = np.arange(P)[:, None]   # k row
    qr = np.arange(P)[None, :]   # q col
    tri_c = np.where(kr > qr, NEGC, 0.0).astype(np.float32)   # [k, q]
    negc = np.full((P, P), NEGC, np.float32)

    cos_h, sin_h, bmask_h, trim_h, dtab_h = [], [], [], [], []
    for h in (0, 1):
        order = _roll_order(h)
        colidx = (order[:, None] * P + np.arange(P)[None, :]).reshape(-1)
        cos_h.append(np.ascontiguousarray(cos2[:, colidx]))
        sin_h.append(np.ascontiguousarray(sin2[:, colidx]))

        bm = np.empty((32, P, P), np.float32)
        for p in range(32):
            if p % 2 == 0:
                bm[p] = tri_b
            else:
                bm[p] = zb if h == 0 else negb
        bmask_h.append(np.ascontiguousarray(bm.transpose(1, 0, 2)))

        tr = np.concatenate([tri_c, zb if h == 0 else negc], axis=1)
        trim_h.append(np.ascontiguousarray(tr.astype(np.float32)))

        qb4 = (np.arange(S) // P) % 4
        d0 = ((qb4 == 1) if h == 1 else np.zeros(S, bool))
        d1 = (qb4 <= 1)
        dtab_h.append(np.ascontiguousarray(
            np.stack([d0, d1]).astype(np.float16)))

    r2 = (np.arange(S // 2) // P) % 2
    ksel = np.stack([np.where(r2 == 0, NEGC, 0.0),
                     np.where(r2 == 1, NEGC, 0.0)]).astype(np.float16)

    xts = []
    x64 = np.asarray(x, np.float64)
    for h in (0, 1):
        order = _roll_order(h)
        xr = x64.transpose(0, 2, 1).reshape(B, 8, 128, 32, 128)[:, :, :, order, :]
        xr = xr.reshape(B, 8, 128, S)
        xh = xr.astype(np.float16)
        xl = (xr - xh.astype(np.float64)).astype(np.float16)
        xts.append((np.ascontiguousarray(xh), np.ascontiguousarray(xl)))
    return dict(wpk=wpk, p128=p128, cos_h=cos_h, sin_h=sin_h, bmask_h=bmask_h,
                trim_h=trim_h, dtab_h=dtab_h, ksel=ksel, xts=xts)


def _in_maps(prep):
    ms = []
    for c in range(8):
        b, h = c // 2, c % 2
        xh, xl = prep["xts"][h]
        ms.append({
            "xth": xh[b], "xtl": xl[b], "wpk": prep["wpk"],
            "cosr": prep["cos_h"][h], "sinr": prep["sin_h"][h],
            "p128": prep["p128"], "bmaski": prep["bmask_h"][h],
            "trimi": prep["trim_h"][h], "dtabi": prep["dtab_h"][h],
            "kseli": prep["ksel"],
        })
    return ms


def kernel(x, mask, W_Q, W_K, W_V):
    x = np.asarray(x, np.float32)
    mask = np.asarray(mask)
    if not np.array_equal(mask, np.tril(np.ones((S, S), mask.dtype))):
        Q = x @ W_Q
        K = x @ W_K
        V = x @ W_V
        pos = np.arange(S)
        inv = 1.0 / (10000.0 ** (2.0 * np.arange(32) / 64))
        th = pos[:, None] * inv[None, :]
        sn, cs = np.sin(th), np.cos(th)

        def rp(q):
            x1, x2 = q[..., 0::2], q[..., 1::2]
            o = np.empty_like(q)
            o[..., 0::2] = x1 * cs - x2 * sn
            o[..., 1::2] = x1 * sn + x2 * cs
            return o
        Q, K = rp(Q), rp(K)
        s = np.einsum('bqd,bkd->bqk', Q, K) / 8.0
        s = np.where(mask == 0, -np.inf, s)
        e = np.exp(s - s.max(-1, keepdims=True))
        return (np.einsum('bqk,bkd->bqd', e / e.sum(-1, keepdims=True), V)
                ).astype(np.float32)

    from concourse.bass_utils import run_bass_kernel_spmd
    if "nc" not in _CACHE:
        _CACHE["nc"] = _build_program()
    nc = _CACHE["nc"]

    prep = _host_prep(x, W_Q, W_K, W_V)
    res = run_bass_kernel_spmd(nc, _in_maps(prep), core_ids=list(range(8)))

    out = np.empty((B, S, DH), np.float32)
    for b in range(B):
        oa, mm = [], []
        for h in (0, 1):
            rr = res.results[2 * b + h]
            order = _roll_order(h)
            unroll = np.empty(32, np.int64)
            unroll[order] = np.arange(32)   # global block -> rolled position
            o_r = rr["o_out"].astype(np.float64).reshape(65, 32, 128)[:, unroll]
            m_r = rr["m_out"].astype(np.float64)[unroll]
            oa.append(o_r.reshape(65, S))
            mm.append(m_r.reshape(S))
        M = np.maximum(mm[0], mm[1])
        w = [np.exp(np.minimum(mm[h] - M, 0.0)) for h in (0, 1)]
        num = oa[0][:64] * w[0][None, :] + oa[1][:64] * w[1][None, :]
        den = oa[0][64] * w[0] + oa[1][64] * w[1]
        out[b] = (num / den[None, :]).T.astype(np.float32)
    return out
